# revision 58
# baseline (speedup 1.0000x reference)
"""Bernstein flow density kernel for Trainium2 (8 NeuronCores, data-parallel).

Math (per sample x in R^5, per dim i):
  c = constrained(A_i)                     # [(4)^i, 15] monotone coeffs in (0,1)
  tf_k = sum_j cb_ij c[j,k]                # cb_i = multivariate Bernstein basis over x[:i]
  dcoef_k = tf_k - tf_{k-1}  (tf_{-1}=0, tf_15=1)
  db_k = 16*comb(15,k) x_i^k (1-x_i)^(15-k)
  f_i = sum_k dcoef_k db_k ;  density = prod_i f_i

Device mapping (v2):
  - cb1/cb2/cb3 built on DVE as fp16 monomials (comb folded into weights),
    ln x_d / ln u_d (hi+lo fp16 split) ride in the same 128-col sub-tile
    block; ONE DMA-xbar transpose per block to basis-major.
  - dim-4's K=256 contraction is re-associated: cb4 = cb3 (x) mono(x3), so
    f4 = sum_{q,a,k} cb3_q W4[(a,q),k] mono_a(x3) db_k(x4)
       = sum_{(a,k)} H[(a,k)] db4x[(a,k)],
    H = cb3 @ W4H (K=64, shares the dims-1-3 matmul: M = 48+64+16 = 128),
    db4x[(a,k)] = mono_a(x3) db_k(x4) = exp-of-linear-in-lns (rides the wlog
    pass, M=128: 48 db13 + 64 db4x + 16 db0-with-W0-folded).
  - per group (512 samples): 2 matmuls (wlog, dcoef/H) + 1 Exp + 1 elementwise
    mul + 1 f-sum matmul; density via exp(sum ln f) as before.
  - act-table pass restricted to the Ln+Exp set: no table reload thrash.
"""

import math
import sys

import numpy as np

for _p in ("/opt/trn_rl_repo", "/root/.axon_site/_ro/trn_rl_repo"):
    if _p not in sys.path:
        sys.path.append(_p)

import concourse.hw_specs as _hw_specs

if not getattr(_hw_specs, "_bf_act_patched", False):
    # Restrict the act-table-load pass to the one set containing BOTH Ln and
    # Exp so alternating Ln/Exp activations don't thrash table reloads
    # (~1.3us each). Emitted ids are remapped to the real act_info.json
    # index after finalize (see build_nc).
    _orig_get_act_tables = _hw_specs.get_activation_tables

    def _patched_get_act_tables(arch):
        tabs = _orig_get_act_tables(arch)
        both = {k: v for k, v in tabs.items() if k == "natural_log_exp_and_others"}
        return both or tabs

    _hw_specs.get_activation_tables = _patched_get_act_tables
    _hw_specs._bf_act_patched = True
    _hw_specs._bf_orig_get_act_tables = _orig_get_act_tables

import concourse.bass as bass
import concourse.tile as tile
from concourse import bacc, mybir
from concourse.bass_utils import run_bass_kernel_spmd

bacc.get_activation_tables = _hw_specs.get_activation_tables


def _fixup_act_table_ids(nc):
    """Rewrite act_func_set_ids from the restricted list back to the real
    act_info.json indices."""
    orig = _hw_specs._bf_orig_get_act_tables
    restricted = list(_hw_specs.get_activation_tables(nc.m.arch))
    full = list(orig(nc.m.arch))
    for b in nc.m.functions[0].blocks:
        for ins in b.instructions:
            if isinstance(ins, mybir.InstLoadActFuncSet):
                ins.act_func_set_id = full.index(restricted[ins.act_func_set_id])


F32 = mybir.dt.float32
F16 = mybir.dt.float16

DIM = 5
TF_DEG = 16
N_FULL = 262144
N_CORES = 8
N_CORE = N_FULL // N_CORES  # 32768
SC = 256.0  # scale folded into dcoef weights to keep fp16 away from subnormals
COMB3 = np.array([1.0, 3.0, 3.0, 1.0])
COMB15 = np.array([math.comb(15, k) for k in range(16)], dtype=np.float64)


# ----------------------------------------------------------------- host consts
def _constrained(A):
    A = A.astype(np.float64)
    sp = np.log1p(np.exp(-np.abs(A))) + np.maximum(A, 0.0)  # softplus, stable
    cs = np.cumsum(sp, axis=1)
    return 2.0 * (1.0 / (1.0 + np.exp(-cs)) - 0.5)


def _dev_perm_scale(i):
    """Map device row p (p = sum_d j_d*4^d, j_0 = x0-digit fastest) to
    reference row (j_0 slowest: ref = sum_d j_d*4^(i-1-d)) + comb scale."""
    rows = 4**i
    ref_idx = np.zeros(rows, dtype=np.int64)
    scale = np.ones(rows)
    for p in range(rows):
        r = 0
        s = 1.0
        for d in range(i):
            jd = (p >> (2 * d)) & 3
            r += jd * 4 ** (i - 1 - d)
            s *= COMB3[jd]
        ref_idx[p] = r
        scale[p] = s
    return ref_idx, scale


def _dcoef_weights(C, combscale):
    """C: [rows,15] device-row-ordered coeffs; returns [rows,16] W with the
    tf-difference folded in: sum_j mono_j * W[j,k] = SC * comb-true dcoef_k."""
    rows = C.shape[0]
    W = np.zeros((rows, 16))
    W[:, 0] = C[:, 0]
    W[:, 1:15] = C[:, 1:15] - C[:, 0:14]
    W[:, 15] = 1.0 - C[:, 14]
    return W * combscale[:, None] * SC


def build_consts(A_list):
    Cs = []
    for i in range(DIM):
        C = _constrained(A_list[i])
        if i == 0:
            Cs.append((C, np.ones(1)))
        else:
            ref_idx, scale = _dev_perm_scale(i)
            Cs.append((C[ref_idx], scale))
    Wd = [_dcoef_weights(Cp, sc) for (Cp, sc) in Cs]

    # out-row map for the wlog/exp pass (128 rows):
    #  0:48   (d,k) d=1..3   row 16(d-1)+k     -> matches dcoef13 cols
    #  48:112 (a,k) dim4     row 48+16a+k      -> matches H cols
    #  112:128 k    dim0     row 112+k (W0 folded into bias; unity cols match)
    LC = np.zeros((10, 128))  # coeffs on [lnx0..4, lnu0..4]
    expb = np.zeros(128)
    w13H = np.zeros((84, 128))

    for d in (1, 2, 3):
        W = Wd[d].copy()
        m = np.max(np.abs(W), axis=0)
        e = np.clip(np.round(np.log2(1024.0 / np.maximum(m, 1e-300))), -10, 40)
        W = W * np.exp2(e)[None, :]
        rowbase = {1: 0, 2: 4, 3: 20}[d]
        w13H[rowbase : rowbase + 4**d, 16 * (d - 1) : 16 * d] = W
        for k in range(16):
            r = 16 * (d - 1) + k
            LC[d, r] = k
            LC[5 + d, r] = 15 - k
            expb[r] = math.log(16.0 * COMB15[k]) - e[k] * math.log(2.0)

    W4H = np.zeros((64, 64))
    for a in range(4):
        for k in range(16):
            W4H[:, 16 * a + k] = Wd[4][64 * a : 64 * a + 64, k]
    m = np.max(np.abs(W4H), axis=0)
    e4 = np.clip(np.round(np.log2(1024.0 / np.maximum(m, 1e-300))), -10, 40)
    W4H = W4H * np.exp2(e4)[None, :]
    w13H[20:84, 48:112] = W4H
    for a in range(4):
        for k in range(16):
            r = 48 + 16 * a + k
            LC[3, r] = a  # mono_a(x3) = x3^a u3^(3-a); its comb lives in W4H
            LC[8, r] = 3 - a
            LC[4, r] = k
            LC[9, r] = 15 - k
            expb[r] = math.log(16.0 * COMB15[k]) - e4[16 * a + k] * math.log(2.0)

    W0 = Wd[0][0]
    assert np.all(W0 > 0)
    for k in range(16):
        r = 112 + k
        LC[0, r] = k
        LC[5, r] = 15 - k
        expb[r] = math.log(16.0 * COMB15[k] * W0[k])

    # unity cols 112:128 read cb2 rows (cbA rows 4:20): sum = 1 exactly
    for j1 in range(4):
        for j0 in range(4):
            w13H[4 + 4 * j1 + j0, 112:128] = COMB3[j1] * COMB3[j0]

    wlog1 = np.zeros((128, 128))
    wlog1[84:94, :] = LC  # ln hi rows
    wlog1[94:104, :] = LC  # ln lo rows

    # fsum weights: 8 out rows per 512-sample group (5 f-values + 3
    # positive pads) -> 16 groups pack one [128,512] fpsum tile exactly,
    # so Ln/lnones/exp/store run once per 4 superblocks instead of 1.
    fw8 = np.zeros((128, 8))
    fw8[112:128, 0] = 1.0
    for d in (1, 2, 3):
        fw8[16 * (d - 1) : 16 * d, d] = 1.0
    fw8[48:112, 4] = 1.0
    fw8[:, 5:8] = fw8[:, 0:1]  # pad cols = f0 (positive): Ln-safe rows
    # PSUM matmul outs are 32-partition aligned: each 32-row strip gets 4
    # groups via 4 ACCUMULATING matmuls with shifted variants (slot j puts
    # the 8 cols at out rows 8j, zeros elsewhere add nothing).
    fw = np.zeros((128, 128))
    for j in range(4):
        fw[:, 32 * j + 8 * j : 32 * j + 8 * j + 8] = fw8

    lnones = np.zeros((128, 16))
    for lsb2 in range(4):
        for tp in range(4):
            g = 4 * lsb2 + tp
            lnones[32 * tp + 8 * lsb2 : 32 * tp + 8 * lsb2 + 5, g] = 1.0
    fbias = np.zeros((16, 1))  # 1/SC folded into the Ln scale

    return {
        "w13H": w13H.astype(np.float16),
        "wlog1": wlog1.astype(np.float16),
        "expb": expb.reshape(128, 1).astype(np.float32),
        "fw": fw.astype(np.float16),
        "lnones": lnones.astype(np.float16),
        "fbias": fbias.astype(np.float32),
    }


# ---------------------------------------------------------------- device build
def _ap(t, extra_offset, dims):
    """Manual AP over a tile: keep its partition dim, custom free dims."""
    return bass.AP(
        tensor=t.tensor, offset=t.offset + extra_offset, ap=[list(t.ap[0])] + dims
    )


def build_nc(ncore, nblk, reps=1, psum_bufs=(2, 2, 2, 2), lag_tail=True,
             look=None, interleave=True):
    """nblk = sub-tiles (128 samples each) per block; must be mult of 16.
    reps: repeat the whole kernel body (incl. const loads) — used by test.py
    to measure marginal per-execution HW time by differencing."""
    assert nblk % 16 == 0
    nsamp_blk = 128 * nblk
    assert ncore % nsamp_blk == 0
    nblocks = ncore // nsamp_blk
    ngroups = nblk // 4  # 512-sample groups per block
    xcols = ncore // 128 * DIM

    nc = bacc.Bacc("TRN2", target_bir_lowering=False, debug=False, num_devices=N_CORES)
    xt = nc.declare_dram_parameter("xt", [128, xcols], F32, isOutput=False)
    w13H = nc.declare_dram_parameter("w13H", [84, 128], F16, isOutput=False)
    wlog1 = nc.declare_dram_parameter("wlog1", [128, 128], F16, isOutput=False)
    expb = nc.declare_dram_parameter("expb", [128, 1], F32, isOutput=False)
    fw = nc.declare_dram_parameter("fw", [128, 128], F16, isOutput=False)
    lnones = nc.declare_dram_parameter("lnones", [128, 16], F16, isOutput=False)
    fbias = nc.declare_dram_parameter("fbias", [16, 1], F32, isOutput=False)
    dens = nc.declare_dram_parameter("dens", [ncore], F32, isOutput=True)

    Exp = mybir.ActivationFunctionType.Exp
    Ln = mybir.ActivationFunctionType.Ln

    with tile.TileContext(nc) as tc:
        with (
            tc.tile_pool(name="wc", bufs=1) as wc,
            tc.tile_pool(name="la", bufs=4) as la,
            tc.tile_pool(name="tr", bufs=4) as tr,
            tc.tile_pool(name="gdb", bufs=3) as gdb,
            tc.tile_pool(name="gpr", bufs=9) as gpr,
            tc.tile_pool(name="sb", bufs=2) as sbp,
            tc.tile_pool(name="psg", bufs=psum_bufs[0], space="PSUM") as psg,
            tc.tile_pool(name="psh", bufs=psum_bufs[1], space="PSUM") as psh,
            tc.tile_pool(name="psf", bufs=psum_bufs[2], space="PSUM") as psf,
            tc.tile_pool(name="psd", bufs=psum_bufs[3], space="PSUM") as psd,
        ):
          for _rep in range(reps):
            w13Hsb = wc.tile([84, 128], F16, tag="w13H")
            wlogsb = wc.tile([128, 128], F16, tag="wlog")
            expbsb = wc.tile([128, 1], F32, tag="expb")
            fwsb = wc.tile([128, 128], F16, tag="fw")
            lnosb = wc.tile([128, 16], F16, tag="lno")
            fbsb = wc.tile([16, 1], F32, tag="fb")
            xall = wc.tile([128, xcols], F32, tag="xall")
            x0cols = nblk * 5  # unit 0's x slice: first, to unblock build(0)
            nc.sync.dma_start(out=xall[:, 0:x0cols], in_=xt[:, 0:x0cols])
            for dst, src_ in (
                (w13Hsb, w13H),
                (wlogsb, wlog1),
                (expbsb, expb),
                (fwsb, fw),
                (lnosb, lnones),
                (fbsb, fbias),
            ):
                nc.sync.dma_start(out=dst[:], in_=src_[:])
            nc.sync.dma_start(out=xall[:, x0cols:], in_=xt[:, x0cols:])

            # fpsum packing: 4 superblocks x 4 tp-groups x 8 rows = 128 rows
            # per tile; tail (Ln + lnones + exp + store) runs once per tile.
            fstate = {"tile": None}

            def fsums(work):
                S, prods = work
                lsb2 = S % 4
                if lsb2 == 0:
                    fpsum = psf.tile([128, 512], F32, tag="fpsum")
                    fstate["tile"] = fpsum
                fpsum = fstate["tile"]
                for tp in range(4):
                    nc.tensor.matmul(
                        out=fpsum[32 * tp : 32 * tp + 32, :],
                        lhsT=fwsb[:, 32 * lsb2 : 32 * lsb2 + 32],
                        rhs=prods[tp][:],
                        start=(lsb2 == 0),
                        stop=(lsb2 == 3),
                        tile_position=(0, 32 * tp),
                    )
                if lsb2 == 3:
                    tail(fpsum, S // 4)

            def tail(fpsum, tile_idx):
                lnf = sbp.tile([128, 512], F16, tag="lnf")
                nc.scalar.activation(
                    out=lnf[:], in_=fpsum[:], func=Ln, scale=1.0 / SC
                )
                lnden = psd.tile([16, 512], F32, tag="lnden")
                nc.tensor.matmul(
                    out=lnden[:], lhsT=lnosb[:], rhs=lnf[:], start=True, stop=True
                )
                dens_sb = sbp.tile([16, 512], F32, tag="dens_sb")
                nc.scalar.activation(
                    out=dens_sb[:], in_=lnden[:], func=Exp, bias=fbsb[:]
                )
                base = tile_idx * 8192
                nc.sync.dma_start(
                    out=dens[base : base + 8192].rearrange("(g s) -> g s", g=16),
                    in_=dens_sb[:],
                )

            n = nblk  # sub-tiles per build unit
            nunits = ncore // (128 * n)
            spu = n // 16  # superblocks (2048 samples) per build unit

            def build(u_idx):
                """DVE/ACT basis build + xbar transpose for unit u_idx;
                returns the basis-major cbTA tile."""
                xa = xall[:, u_idx * n * 5 : (u_idx + 1) * n * 5].rearrange(
                    "p (n d) -> p n d", d=5
                )
                x3 = xa[:, :, 0:3]
                u = la.tile([128, n, 3], F32, tag="u")
                xp2 = la.tile([128, n, 3], F32, tag="xp2")
                up2 = la.tile([128, n, 3], F32, tag="up2")
                ln32 = la.tile([128, n, 10], F32, tag="ln32")
                b4 = la.tile([128, n, 4, 3], F16, tag="b4")
                # cbA cols: 0:4 cb1 | 4:20 cb2 | 20:84 cb3 | 84:94 ln hi |
                # 94:104 ln lo | 104:128 junk (never read after transpose)
                cbA = la.tile([128, n, 128], F16, tag="cbA")

                nc.vector.tensor_scalar(
                    out=u[:],
                    in0=x3,
                    scalar1=1.0,
                    scalar2=-1.0,
                    op0=mybir.AluOpType.subtract,
                    op1=mybir.AluOpType.mult,
                )
                nc.vector.tensor_mul(out=xp2[:], in0=x3, in1=x3)
                nc.vector.tensor_mul(out=up2[:], in0=u[:], in1=u[:])
                nc.scalar.activation(out=ln32[:, :, 0:5], in_=xa, func=Ln)
                nc.scalar.activation(
                    out=ln32[:, :, 5:10], in_=xa, func=Ln, scale=-1.0, bias=1.0
                )
                nc.vector.tensor_copy(out=cbA[:, :, 84:94], in_=ln32[:])
                nc.vector.tensor_sub(
                    out=cbA[:, :, 94:104], in0=ln32[:], in1=cbA[:, :, 84:94]
                )
                # b4[:, :, j, d]: j0=u^3, j1=x u^2, j2=x^2 u, j3=x^3 (d=0..2)
                nc.vector.tensor_mul(out=b4[:, :, 0, :], in0=up2[:], in1=u[:])
                nc.vector.tensor_mul(out=b4[:, :, 1, :], in0=x3, in1=up2[:])
                nc.vector.tensor_mul(out=b4[:, :, 2, :], in0=xp2[:], in1=u[:])
                nc.vector.tensor_mul(out=b4[:, :, 3, :], in0=xp2[:], in1=x3)
                # cb1[j0] = mono_{j0}(x0)
                nc.vector.tensor_copy(
                    out=cbA[:, :, 0:4], in_=_ap(b4[:], 0, [[12, n], [3, 4]])
                )
                # cb2[4 j1 + j0] = mono_{j1}(x1) mono_{j0}(x0)
                nc.gpsimd.tensor_mul(
                    out=cbA[:, :, 4:20].rearrange("p n (a b) -> p n a b", a=4),
                    in0=_ap(b4[:], 0, [[12, n], [0, 4], [3, 4]]),
                    in1=_ap(b4[:], 1, [[12, n], [3, 4], [0, 4]]),
                )
                # cb3[16 j2 + (4 j1 + j0)] = mono_{j2}(x2) cb2 (in halves:
                # shorter Pool ops pipeline better with the transposes)
                hn = n // 2
                for hh in range(2):
                    nc.gpsimd.tensor_mul(
                        out=cbA[:, hh * hn : (hh + 1) * hn, 20:84].rearrange(
                            "p n (a b) -> p n a b", a=4
                        ),
                        in0=_ap(
                            cbA[:], 128 * hh * hn + 4, [[128, hn], [0, 4], [1, 16]]
                        ),
                        in1=_ap(b4[:], 12 * hh * hn + 2, [[12, hn], [3, 4], [0, 16]]),
                    )
                # xbar transposes (per half-unit, so the first superblock
                # of the unit unblocks sooner):
                # cbTA[:, j, :] = transpose(cbA[:, j128:(j+1)128])
                cbTA = tr.tile([128, n, 128], F16, tag="cbTA")
                h = n // 2
                for hh in range(2):
                    nc.sync.dma_start(
                        out=cbTA[:, hh * h : (hh + 1) * h, :],
                        in_=cbA[:, hh * h : (hh + 1) * h, :].rearrange(
                            "p n c -> p (n c)"
                        ),
                        transpose=True,
                    )
                return cbTA

            def groups(sb_idx, cbTA, lsb):
                prods = []
                for tp in range(4):
                    gsl = slice(16 * lsb + 4 * tp, 16 * lsb + 4 * tp + 4)
                    wlogp = psg.tile([128, 512], F32, tag="wlogp")
                    nc.tensor.matmul(
                        out=wlogp[:],
                        lhsT=wlogsb[0:104, :],
                        rhs=cbTA[0:104, gsl, :],
                        start=True,
                        stop=True,
                    )
                    dtfp = psh.tile([128, 512], F32, tag="dtfp")
                    nc.tensor.matmul(
                        out=dtfp[:],
                        lhsT=w13Hsb[:],
                        rhs=cbTA[0:84, gsl, :],
                        start=True,
                        stop=True,
                    )
                    dbT = gdb.tile([128, 512], F16, tag="dbT")
                    nc.scalar.activation(
                        out=dbT[:], in_=wlogp[:], func=Exp, bias=expbsb[:]
                    )
                    prod = gpr.tile([128, 512], F16, tag="prod")
                    # gpsimd cannot read PSUM (dtfp): prods stay on DVE
                    nc.vector.tensor_mul(out=prod[:], in0=dtfp[:], in1=dbT[:])
                    prods.append(prod)
                return (sb_idx, prods)

            # software pipeline over superblocks: iteration S issues
            # build(unit+LOOK) at unit starts, groups(S), fsums(S-1),
            # tail(S-2) — every queued op's inputs are already >= 1
            # iteration in flight, so no queue head-blocks.
            LOOK = look if look is not None else max(1, 32 // n)
            nsb = nunits * spu
            cbTAs = {}
            for u_idx in range(min(LOOK, nunits)):
                cbTAs[u_idx] = build(u_idx)
            pend_f = None
            for S in range(nsb):
                u_idx, lsb = divmod(S, spu)
                work = groups(S, cbTAs[u_idx], lsb)
                if lsb == spu - 1:
                    del cbTAs[u_idx]
                    if u_idx + LOOK < nunits:
                        cbTAs[u_idx + LOOK] = build(u_idx + LOOK)
                if not lag_tail:
                    fsums(work)
                    continue
                if pend_f is not None:
                    fsums(pend_f)
                pend_f = work
            if lag_tail and pend_f is not None:
                fsums(pend_f)
    nc.finalize()
    _fixup_act_table_ids(nc)
    return nc


# -------------------------------------------------------------------- host run
def pack_x(x_shard):
    """[N_CORE, 5] -> [128, N_CORE/128*5]; sample s = nb*128+p -> row p, cols nb*5+d."""
    n = x_shard.shape[0]
    return (
        np.ascontiguousarray(x_shard.reshape(n // 128, 128, 5).transpose(1, 0, 2))
        .reshape(128, n // 128 * 5)
        .astype(np.float32)
    )


_CACHE = {}


def _get_runner(reps=1):
    """Build nc + a cached jitted shard_map callable (trace/compile once)."""
    key = ("runner", reps)
    if key in _CACHE:
        return _CACHE[key]
    import jax
    from jax.sharding import Mesh, PartitionSpec
    from jax.experimental.shard_map import shard_map

    from concourse import bass2jax, mybir as _mb
    from concourse.bass2jax import (
        _bass_exec_p,
        install_neuronx_cc_hook,
        partition_id_tensor,
    )

    install_neuronx_cc_hook()
    nc = build_nc(N_CORE, 32, reps=reps, look=4)
    partition_name = nc.partition_id_tensor.name if nc.partition_id_tensor else None

    in_names, out_names, out_avals, zero_outs = [], [], [], []
    for alloc in nc.m.functions[0].allocations:
        if not isinstance(alloc, _mb.MemoryLocationSet):
            continue
        name = alloc.memorylocations[0].name
        if alloc.kind == "ExternalInput":
            if name != partition_name:
                in_names.append(name)
        elif alloc.kind == "ExternalOutput":
            out_names.append(name)
            shape = tuple(alloc.tensor_shape)
            dtype = _mb.dt.np(alloc.dtype)
            out_avals.append(jax.core.ShapedArray(shape, dtype))
            zero_outs.append(np.zeros(shape, dtype))
    n_params = len(in_names)
    all_in_names = list(in_names) + list(out_names)
    if partition_name is not None:
        all_in_names.append(partition_name)

    def _body(*args):
        operands = list(args)
        if partition_name is not None:
            operands.append(partition_id_tensor())
        outs = _bass_exec_p.bind(
            *operands,
            out_avals=tuple(out_avals),
            in_names=tuple(all_in_names),
            out_names=tuple(out_names),
            lowering_input_output_aliases=(),
            sim_require_finite=True,
            sim_require_nnan=True,
            nc=nc,
        )
        return tuple(outs)

    devices = jax.devices()[:N_CORES]
    mesh = Mesh(np.asarray(devices), ("core",))
    in_specs = (PartitionSpec("core"),) * (n_params + len(out_names))
    out_specs = (PartitionSpec("core"),) * len(out_names)
    sharded = jax.jit(
        shard_map(
            _body, mesh=mesh, in_specs=in_specs, out_specs=out_specs, check_rep=False
        ),
        keep_unused=True,
    )
    shard = jax.NamedSharding(mesh, PartitionSpec("core"))
    zeros_dev = [
        jax.device_put(
            np.zeros((N_CORES * z.shape[0], *z.shape[1:]), z.dtype), shard
        )
        for z in zero_outs
    ]
    _CACHE[key] = (sharded, in_names, out_names, out_avals, zeros_dev, shard)
    return _CACHE[key]


def run_device(in_maps):
    """in_maps: per-core dicts. Returns list of per-core output dicts."""
    import jax

    sharded, in_names, out_names, out_avals, zeros_dev, shard = _get_runner()
    concat_in = [
        jax.device_put(
            np.concatenate(
                [np.asarray(in_maps[c][k]) for c in range(N_CORES)], axis=0
            ),
            shard,
        )
        for k in in_names
    ]
    out_arrs = sharded(*concat_in, *zeros_dev)
    return [
        {
            k: np.asarray(out_arrs[i]).reshape(N_CORES, *out_avals[i].shape)[c]
            for i, k in enumerate(out_names)
        }
        for c in range(N_CORES)
    ]


def make_in_maps(x, A_list):
    consts = build_consts([np.asarray(a) for a in A_list])
    in_maps = []
    for c in range(N_CORES):
        m = {"xt": pack_x(x[c * N_CORE : (c + 1) * N_CORE])}
        m.update(consts)
        in_maps.append(m)
    return in_maps


def kernel(x, A0, A1, A2, A3, A4):
    x = np.asarray(x, dtype=np.float32)
    in_maps = make_in_maps(x, (A0, A1, A2, A3, A4))
    res = run_device(in_maps)
    return np.concatenate([res[c]["dens"] for c in range(N_CORES)])
